# revision 5
# baseline (speedup 1.0000x reference)
"""Trainium2 Bass kernel: depthwise 3x3 stencil conv (SAME, zero-pad) + residual.

Math (per image, per channel):
    out[h,w] = sum_{dh,dw} k[dh,dw] * x[h+dh-1, w+dw-1]  +  x[h,w]

The fixed stencil k = [[1,0,-1],[0,1,0],[-1,0,1]] is rank-2:
    k = outer((1,0,-1),(1,0,-1)) + center(1)
so with t[h,w] = x[h-1,w] - x[h+1,w] (vertical pass):
    out[h,w] = 2*x[h,w] + t[h,w-1] - t[h,w+1]

Mapping on one NeuronCore (batch is sharded 4 images/core across 8 cores):
  - layout: partitions = h (112 rows), free dim = (w,c) flattened (10752 f32)
    with 96-float zero pads on both ends (one w column, padded host-side)
  - vertical pass: banded 112x112 matmul on TensorE (PSUM, N=512 chunks)
  - PSUM -> SBUF t-slab copies on ScalarE
  - horizontal pass: two fused in-place DVE ops per piece:
        v   = 2*x + t@(w-1)      (scalar_tensor_tensor)
        out = v - t@(w+1)        (tensor_tensor)
  - straight contiguous HBM DMAs in/out (HWDGE)

fp32 self-loading matmuls can carry only ~1 semaphore wait (single EVENTS
slot in the LDWEIGHTS ISA struct), so DMA-completion waits are absorbed by
tiny dummy matmuls that read one column of the freshly loaded tile.
"""

import sys
import numpy as np

for _p in ("/opt/trn_rl_repo",):
    if _p not in sys.path:
        sys.path.insert(0, _p)

# ---------------- problem constants (hardcoded per contract) ----------------
N_CORES = 8
N, H, W, CH = 32, 112, 112, 96
IMGS_PER_CORE = N // N_CORES          # 4
ROWS = IMGS_PER_CORE * H              # 448 rows per core shard
FS = W * CH                           # 10752 floats per row
PAD = CH                              # one w column of zero padding
SLAB = FS + 2 * PAD                   # 10944
MM_N = 512                            # one PSUM bank of fp32
N_PIECES = 3                          # DVE piece split of the interior
PIECE = FS // N_PIECES                # 3584

_CACHE = {}
LAST_RESULTS = None  # BassKernelResults of the most recent run (for test.py)


def _build_bass(beta):
    """Raw-bass program with a hand-rolled static schedule.

    The walrus codegen used on this toolchain supports at most ONE semaphore
    wait per instruction, which rules out Tile's auto-generated multi-wait
    instructions.  Raw bass emits each wait as its own standalone wait_ge
    instruction on the consuming engine, which is always legal.

    Per-image dataflow (i = 0..3, double-buffered xt/tt slabs):
        SP :  D(i)  x rows -> xt[i%2]                (HBM -> SBUF)
        PE :  mm(i,g) ps[bank] = V^T @ xt[:, g]      (vertical pass, 22 groups)
        ACT:  cp(i,g) tt[:, g] <- ps[bank]           (PSUM -> SBUF)
        DVE:  op1_p  xt[a:b] = beta*xt[a:b] + tt[a-96:b-96]   (p = 0..2)
              op2_p  tt[a:b] = xt[a:b] - tt[a+96:b+96]
              drain  -> inc dve sem
        SP :  O(i)  tt[:, 96:10848] -> out rows      (SBUF -> HBM)
    """
    from concourse import bass, mybir

    f32 = mybir.dt.float32
    nc = bass.Bass(debug=False)
    x_d = nc.declare_dram_parameter("x", [ROWS, SLAB], f32, isOutput=False)
    v_d = nc.declare_dram_parameter("vmat", [H, H], f32, isOutput=False)
    out_d = nc.declare_dram_parameter("out", [ROWS, FS], f32, isOutput=True)

    groups = []
    off = 0
    while off < SLAB:
        n = min(MM_N, SLAB - off)
        groups.append((off, n))
        off += n
    n_g = len(groups)  # 22

    vt = nc.alloc_sbuf_tensor("vt", [H, H], f32)
    xt = [nc.alloc_sbuf_tensor(f"xt{k}", [H, SLAB], f32) for k in range(2)]
    tt = [nc.alloc_sbuf_tensor(f"tt{k}", [H, SLAB], f32) for k in range(2)]
    NB = 6
    ps = [nc.alloc_psum_tensor(f"ps{b}", [H, MM_N], f32) for b in range(NB)]

    # DVE piece boundaries (a, b) in slab coordinates
    pieces = [(PAD + p * PIECE, PAD + (p + 1) * PIECE) for p in range(N_PIECES)]

    def groups_through(elem):
        """number of leading matmul groups needed to cover slab[0:elem)"""
        return min(n_g, (elem + MM_N - 1) // MM_N)

    with (
        nc.Block() as block,
        nc.semaphore("s_vt") as s_vt,
        nc.semaphore("s_din") as s_din,
        nc.semaphore("s_pe") as s_pe,
        nc.semaphore("s_act") as s_act,
        nc.semaphore("s_dve") as s_dve,
        nc.semaphore("s_dout") as s_dout,
    ):

        @block.sync
        def _(sp: bass.BassEngine):
            sp.dma_start(out=vt[:, :], in_=v_d[:, :]).then_inc(s_vt, 16)
            for i in range(min(2, IMGS_PER_CORE)):
                sp.dma_start(
                    out=xt[i % 2][:, :], in_=x_d[i * H : (i + 1) * H, :]
                ).then_inc(s_din, 16)
            for i in range(IMGS_PER_CORE):
                # store image i once its DVE drain fired
                sp.wait_ge(s_dve, i + 1)
                sp.dma_start(
                    out=out_d[i * H : (i + 1) * H, :],
                    in_=tt[i % 2][:, PAD : PAD + FS],
                ).then_inc(s_dout, 16)
                nxt = i + 2
                if nxt < IMGS_PER_CORE:
                    # reload xt[i%2]: all PE reads (mm) and DVE accesses of
                    # image i must be done (DVE covered by the wait above)
                    sp.wait_ge(s_pe, n_g * (i + 1))
                    sp.dma_start(
                        out=xt[nxt % 2][:, :], in_=x_d[nxt * H : (nxt + 1) * H, :]
                    ).then_inc(s_din, 16)
            sp.wait_ge(s_dout, 16 * IMGS_PER_CORE)

        @block.tensor
        def _(pe: bass.BassEngine):
            pe.wait_ge(s_vt, 16)
            for i in range(IMGS_PER_CORE):
                pe.wait_ge(s_din, 16 * (i + 1))
                for g, (goff, gn) in enumerate(groups):
                    idx = i * n_g + g
                    if idx >= NB:
                        # psum bank reuse: the copy that read it must be done
                        pe.wait_ge(s_act, idx - NB + 1)
                    pe.matmul(
                        out=ps[idx % NB][0:H, 0:gn],
                        lhsT=vt[:, :],
                        rhs=xt[i % 2][:, goff : goff + gn],
                        start=True,
                        stop=True,
                    ).then_inc(s_pe, 1)

        @block.scalar
        def _(act: bass.BassEngine):
            for i in range(IMGS_PER_CORE):
                if i >= 2:
                    # tt[i%2] slab reuse: image i-2's DVE ops and its store
                    # DMA must both be finished with it
                    act.wait_ge(s_dve, i - 1)
                    act.wait_ge(s_dout, 16 * (i - 1))
                for g, (goff, gn) in enumerate(groups):
                    idx = i * n_g + g
                    act.wait_ge(s_pe, idx + 1)
                    act.copy(
                        out=tt[i % 2][:, goff : goff + gn],
                        in_=ps[idx % NB][0:H, 0:gn],
                    ).then_inc(s_act, 1)

        @block.vector
        def _(dve: bass.BassEngine):
            for i in range(IMGS_PER_CORE):
                dve.wait_ge(s_din, 16 * (i + 1))
                # op1 pieces first (they consume original t values), then op2
                for p, (a, b) in enumerate(pieces):
                    # WAR: op1 overwrites xt[a:b) which matmul groups read
                    dve.wait_ge(s_pe, i * n_g + groups_through(b))
                    # RAW: needs tt[a-96 : b-96)
                    dve.wait_ge(s_act, i * n_g + groups_through(b - PAD))
                    dve.scalar_tensor_tensor(
                        out=xt[i % 2][:, a:b],
                        in0=xt[i % 2][:, a:b],
                        scalar=float(beta),
                        in1=tt[i % 2][:, a - PAD : b - PAD],
                        op0=mybir.AluOpType.mult,
                        op1=mybir.AluOpType.add,
                    )
                for p, (a, b) in enumerate(pieces):
                    dve.wait_ge(s_act, i * n_g + groups_through(b + PAD))
                    dve.tensor_tensor(
                        out=tt[i % 2][:, a:b],
                        in0=xt[i % 2][:, a:b],
                        in1=tt[i % 2][:, a + PAD : b + PAD],
                        op=mybir.AluOpType.subtract,
                    )
                dve.drain().then_inc(s_dve, 1)

    return nc


def _stencil_params(kern):
    """Validate the depthwise kernel and extract (vertical profile a, beta).

    Requires: channels identical, k[:,2] == -k[:,0], k[0,1] == k[2,1] == 0.
    Returns (a, beta) with a = k[:,0] (vertical mixing profile) and
    beta = k[1,1] + 1 (center coefficient incl. the residual).
    """
    k = np.asarray(kern, dtype=np.float32)
    if k.ndim != 4 or k.shape != (3, 3, 1, CH):
        return None
    if not np.all(k == k[:, :, :, :1]):
        return None
    k2 = k[:, :, 0, 0]
    if not (np.all(k2[:, 2] == -k2[:, 0]) and k2[0, 1] == 0 and k2[2, 1] == 0):
        return None
    return k2[:, 0].copy(), float(k2[1, 1]) + 1.0


def _numpy_fallback(x, kern):
    """Straightforward shifted-add implementation (safety net only)."""
    k = np.asarray(kern, dtype=np.float32)[:, :, 0, :]  # (3,3,CH)
    xp = np.pad(x, ((0, 0), (1, 1), (1, 1), (0, 0)))
    out = x.astype(np.float32).copy()
    for dh in range(3):
        for dw in range(3):
            out += k[dh, dw] * xp[:, dh : dh + H, dw : dw + W, :]
    return out


def _ensure_ntff_hook():
    """The agent image's antenv lacks axon_hooks; synthesize it so
    run_bass_kernel_spmd(trace=True) can reach the NTFF profiler."""
    import types

    if "antenv.axon_hooks" in sys.modules:
        return
    import antenv

    mod = types.ModuleType("antenv.axon_hooks")
    state = {}
    mod.set_axon_ntff_profile_hook = lambda h: state.__setitem__("h", h)
    mod.get_axon_ntff_profile_hook = lambda: state.get("h")
    sys.modules["antenv.axon_hooks"] = mod
    antenv.axon_hooks = mod
    try:
        if "/root/.axon_site" not in sys.path:
            sys.path.insert(0, "/root/.axon_site")
        from trn_agent_boot.trn_boot import _ntff_profile_via_ctypes

        hook = _ntff_profile_via_ctypes("/opt/axon/libaxon_pjrt.so")
        if hook is not None:
            mod.set_axon_ntff_profile_hook(hook)
    except Exception:
        pass


def _run_on_hw(x, a, beta, trace=False):
    global LAST_RESULTS
    if trace:
        _ensure_ntff_hook()
    from concourse.bass_utils import run_bass_kernel_spmd

    # vertical banded matrix: V[i, j] = coeff of x-row i in t-row j
    V = np.zeros((H, H), dtype=np.float32)
    idx = np.arange(H)
    V[idx[:-1] + 1, idx[:-1]] += a[2]   # i = j+1
    V[idx, idx] += a[1]                 # i = j
    V[idx[1:] - 1, idx[1:]] += a[0]     # i = j-1

    key = (a.tobytes(), float(beta))
    if key not in _CACHE:
        _CACHE[key] = _build_bass(beta)
    nc = _CACHE[key]

    # host-side zero padding of one w column on each side (pads the slab so
    # the device needs no memsets)
    xp = np.zeros((N_CORES, ROWS, SLAB), dtype=np.float32)
    xp[:, :, PAD : PAD + FS] = x.reshape(N_CORES, ROWS, FS)
    in_maps = [{"x": xp[c], "vmat": V} for c in range(N_CORES)]
    res = run_bass_kernel_spmd(nc, in_maps, list(range(N_CORES)), trace=trace)
    LAST_RESULTS = res
    out = np.stack([res.results[c]["out"] for c in range(N_CORES)])
    return out.reshape(N, H, W, CH)


def kernel(x, kernel=None, _trace=False, **_unused):
    x = np.ascontiguousarray(np.asarray(x, dtype=np.float32))
    assert x.shape == (N, H, W, CH), f"unexpected x shape {x.shape}"
    if kernel is None:
        base = np.array(
            [[1.0, 0.0, -1.0], [0.0, 1.0, 0.0], [-1.0, 0.0, 1.0]], dtype=np.float32
        )
        kernel = np.tile(base[:, :, None, None], (1, 1, 1, CH))
    params = _stencil_params(kernel)
    if params is None:
        return _numpy_fallback(x, kernel)
    a, beta = params
    return _run_on_hw(x, a, beta, trace=_trace)


if __name__ == "__main__":
    xs = np.random.randn(N, H, W, CH).astype(np.float32)
    out = kernel(xs)
    print(out.shape, out.dtype)


# revision 6
# speedup vs baseline: 1.0109x; 1.0109x over previous
"""Trainium2 Bass kernel: depthwise 3x3 stencil conv (SAME, zero-pad) + residual.

Math (per image, per channel):
    out[h,w] = sum_{dh,dw} k[dh,dw] * x[h+dh-1, w+dw-1]  +  x[h,w]

The fixed stencil k = [[1,0,-1],[0,1,0],[-1,0,1]] is rank-2:
    k = outer((1,0,-1),(1,0,-1)) + center(1)
so with t[h,w] = x[h-1,w] - x[h+1,w] (vertical pass):
    out[h,w] = 2*x[h,w] + t[h,w-1] - t[h,w+1]

Mapping on one NeuronCore (batch is sharded 4 images/core across 8 cores):
  - layout: partitions = h (112 rows), free dim = (w,c) flattened (10752 f32)
    with 96-float zero pads on both ends (one w column, padded host-side)
  - vertical pass: banded 112x112 matmul on TensorE (PSUM, N=512 chunks)
  - PSUM -> SBUF t-slab copies on ScalarE
  - horizontal pass: two fused in-place DVE ops per piece:
        v   = 2*x + t@(w-1)      (scalar_tensor_tensor)
        out = v - t@(w+1)        (tensor_tensor)
  - straight contiguous HBM DMAs in/out (HWDGE)

fp32 self-loading matmuls can carry only ~1 semaphore wait (single EVENTS
slot in the LDWEIGHTS ISA struct), so DMA-completion waits are absorbed by
tiny dummy matmuls that read one column of the freshly loaded tile.
"""

import sys
import numpy as np

for _p in ("/opt/trn_rl_repo",):
    if _p not in sys.path:
        sys.path.insert(0, _p)

# ---------------- problem constants (hardcoded per contract) ----------------
N_CORES = 8
N, H, W, CH = 32, 112, 112, 96
IMGS_PER_CORE = N // N_CORES          # 4
ROWS = IMGS_PER_CORE * H              # 448 rows per core shard
FS = W * CH                           # 10752 floats per row
PAD = CH                              # one w column of zero padding
SLAB = FS + 2 * PAD                   # 10944
MM_N = 512                            # one PSUM bank of fp32
N_PIECES = 3                          # DVE piece split of the interior
PIECE = FS // N_PIECES                # 3584

_CACHE = {}
LAST_RESULTS = None  # BassKernelResults of the most recent run (for test.py)


def _build_bass(beta):
    """Raw-bass program with a hand-rolled static schedule.

    The walrus codegen used on this toolchain supports at most ONE semaphore
    wait per instruction, which rules out Tile's auto-generated multi-wait
    instructions.  Raw bass emits each wait as its own standalone wait_ge
    instruction on the consuming engine, which is always legal.

    Per-image dataflow (i = 0..3, double-buffered xt/tt slabs):
        SP :  D(i)  x rows -> xt[i%2]                (HBM -> SBUF)
        PE :  mm(i,g) ps[bank] = V^T @ xt[:, g]      (vertical pass, 22 groups)
        ACT:  cp(i,g) tt[:, g] <- ps[bank]           (PSUM -> SBUF)
        DVE:  op1_p  xt[a:b] = beta*xt[a:b] + tt[a-96:b-96]   (p = 0..2)
              op2_p  tt[a:b] = xt[a:b] - tt[a+96:b+96]
              drain  -> inc dve sem
        SP :  O(i)  tt[:, 96:10848] -> out rows      (SBUF -> HBM)
    """
    from concourse import bass, mybir

    f32 = mybir.dt.float32
    nc = bass.Bass(debug=False)
    x_d = nc.declare_dram_parameter("x", [ROWS, SLAB], f32, isOutput=False)
    v_d = nc.declare_dram_parameter("vmat", [H, H], f32, isOutput=False)
    out_d = nc.declare_dram_parameter("out", [ROWS, FS], f32, isOutput=True)

    groups = []
    off = 0
    while off < SLAB:
        n = min(MM_N, SLAB - off)
        groups.append((off, n))
        off += n
    n_g = len(groups)  # 22

    vt = nc.alloc_sbuf_tensor("vt", [H, H], f32)
    xt = [nc.alloc_sbuf_tensor(f"xt{k}", [H, SLAB], f32) for k in range(2)]
    tt = [nc.alloc_sbuf_tensor(f"tt{k}", [H, SLAB], f32) for k in range(2)]
    NB = 8
    ps = [nc.alloc_psum_tensor(f"ps{b}", [H, MM_N], f32) for b in range(NB)]

    # DVE piece boundaries (a, b) in slab coordinates
    pieces = [(PAD + p * PIECE, PAD + (p + 1) * PIECE) for p in range(N_PIECES)]
    # in-DMA split point (multiple of MM_N): groups 0..10 / 11..21
    DSPLIT = 11 * MM_N  # 5632

    def groups_through(elem):
        """number of leading matmul groups needed to cover slab[0:elem)"""
        return min(n_g, (elem + MM_N - 1) // MM_N)

    with (
        nc.Block(no_gpsimd_drain=True) as block,
        nc.semaphore("s_vt") as s_vt,
        nc.semaphore("s_din") as s_din,
        nc.semaphore("s_pe") as s_pe,
        nc.semaphore("s_act") as s_act,
        nc.semaphore("s_dve") as s_dve,
        nc.semaphore("s_gps") as s_gps,
        nc.semaphore("s_dout") as s_dout,
    ):
        NP = N_PIECES  # 3 out-DMA pieces / DVE ops / gpsimd ops per image

        @block.sync
        def _(sp: bass.BassEngine):
            sp.dma_start(out=vt[:, :], in_=v_d[:, :]).then_inc(s_vt, 16)

            def load(i):
                r0 = i * H
                sp.dma_start(
                    out=xt[i % 2][:, 0:DSPLIT], in_=x_d[r0 : r0 + H, 0:DSPLIT]
                ).then_inc(s_din, 16)
                sp.dma_start(
                    out=xt[i % 2][:, DSPLIT:SLAB], in_=x_d[r0 : r0 + H, DSPLIT:SLAB]
                ).then_inc(s_din, 16)

            load(0)
            load(1)
            for i in range(IMGS_PER_CORE):
                r0 = i * H
                for p in range(NP):
                    # store piece p of image i once gpsimd finished it
                    sp.wait_ge(s_gps, NP * i + p + 1)
                    sp.dma_start(
                        out=out_d[r0 : r0 + H, p * PIECE : (p + 1) * PIECE],
                        in_=tt[i % 2][:, PAD + p * PIECE : PAD + (p + 1) * PIECE],
                    ).then_inc(s_dout, 16)
                nxt = i + 2
                if nxt < IMGS_PER_CORE:
                    # reload xt[i%2]: PE reads, DVE in-place ops and gpsimd
                    # reads of image i must all be done
                    sp.wait_ge(s_pe, n_g * (i + 1))
                    sp.wait_ge(s_dve, NP * (i + 1))
                    load(nxt)
            sp.wait_ge(s_dout, 16 * NP * IMGS_PER_CORE)

        @block.tensor
        def _(pe: bass.BassEngine):
            pe.wait_ge(s_vt, 16)
            for i in range(IMGS_PER_CORE):
                for g, (goff, gn) in enumerate(groups):
                    idx = i * n_g + g
                    if g == 0:
                        pe.wait_ge(s_din, 32 * i + 16)
                    elif g == 11:
                        pe.wait_ge(s_din, 32 * i + 32)
                    if idx >= NB:
                        # psum bank reuse: the copy that read it must be done
                        pe.wait_ge(s_act, idx - NB + 1)
                    pe.matmul(
                        out=ps[idx % NB][0:H, 0:gn],
                        lhsT=vt[:, :],
                        rhs=xt[i % 2][:, goff : goff + gn],
                        start=True,
                        stop=True,
                    ).then_inc(s_pe, 1)

        @block.scalar
        def _(act: bass.BassEngine):
            for i in range(IMGS_PER_CORE):
                if i >= 2:
                    # tt[i%2] slab reuse: image i-2's gpsimd writes (which
                    # follow all its DVE reads) and its store DMAs must be done
                    act.wait_ge(s_gps, NP * (i - 1))
                    act.wait_ge(s_dout, 16 * NP * (i - 1))
                for g, (goff, gn) in enumerate(groups):
                    idx = i * n_g + g
                    act.wait_ge(s_pe, idx + 1)
                    act.copy(
                        out=tt[i % 2][:, goff : goff + gn],
                        in_=ps[idx % NB][0:H, 0:gn],
                    ).then_inc(s_act, 1)

        @block.vector
        def _(dve: bass.BassEngine):
            # op1_p: v = beta*x + t@(w-1), in place on the x slab
            for i in range(IMGS_PER_CORE):
                for p, (a, b) in enumerate(pieces):
                    if p == 0:
                        dve.wait_ge(s_din, 32 * i + 16)
                    elif p == 1:
                        dve.wait_ge(s_din, 32 * i + 32)
                    # WAR: op1 overwrites xt[a:b) which matmul groups read
                    dve.wait_ge(s_pe, i * n_g + groups_through(b))
                    # RAW: needs tt[a-96 : b-96)
                    dve.wait_ge(s_act, i * n_g + groups_through(b - PAD))
                    dve.scalar_tensor_tensor(
                        out=xt[i % 2][:, a:b],
                        in0=xt[i % 2][:, a:b],
                        scalar=float(beta),
                        in1=tt[i % 2][:, a - PAD : b - PAD],
                        op0=mybir.AluOpType.mult,
                        op1=mybir.AluOpType.add,
                    )
                    # drain so gpsimd sees the writes, and count pieces
                    dve.drain().then_inc(s_dve, 1)

        @block.gpsimd
        def _(gps: bass.BassEngine):
            # op2_p: out = v - t@(w+1), written into the tt slab (the final
            # result), consumed by the store DMA.  Must run after ALL op1
            # pieces of the image: op1_{p+1} reads tt[b_p-96 : b_p) which
            # op2_p overwrites.
            for i in range(IMGS_PER_CORE):
                for p, (a, b) in enumerate(pieces):
                    if p == 0:
                        gps.wait_ge(s_dve, NP * (i + 1))
                        gps.wait_ge(s_act, i * n_g + n_g)
                        if i >= 2:
                            # overwrite of tt: store DMAs of image i-2 done
                            gps.wait_ge(s_dout, 16 * NP * (i - 1))
                    gps.tensor_tensor(
                        out=tt[i % 2][:, a:b],
                        in0=xt[i % 2][:, a:b],
                        in1=tt[i % 2][:, a + PAD : b + PAD],
                        op=mybir.AluOpType.subtract,
                    ).then_inc(s_gps, 1)

    return nc


def _stencil_params(kern):
    """Validate the depthwise kernel and extract (vertical profile a, beta).

    Requires: channels identical, k[:,2] == -k[:,0], k[0,1] == k[2,1] == 0.
    Returns (a, beta) with a = k[:,0] (vertical mixing profile) and
    beta = k[1,1] + 1 (center coefficient incl. the residual).
    """
    k = np.asarray(kern, dtype=np.float32)
    if k.ndim != 4 or k.shape != (3, 3, 1, CH):
        return None
    if not np.all(k == k[:, :, :, :1]):
        return None
    k2 = k[:, :, 0, 0]
    if not (np.all(k2[:, 2] == -k2[:, 0]) and k2[0, 1] == 0 and k2[2, 1] == 0):
        return None
    return k2[:, 0].copy(), float(k2[1, 1]) + 1.0


def _numpy_fallback(x, kern):
    """Straightforward shifted-add implementation (safety net only)."""
    k = np.asarray(kern, dtype=np.float32)[:, :, 0, :]  # (3,3,CH)
    xp = np.pad(x, ((0, 0), (1, 1), (1, 1), (0, 0)))
    out = x.astype(np.float32).copy()
    for dh in range(3):
        for dw in range(3):
            out += k[dh, dw] * xp[:, dh : dh + H, dw : dw + W, :]
    return out


def _ensure_ntff_hook():
    """The agent image's antenv lacks axon_hooks; synthesize it so
    run_bass_kernel_spmd(trace=True) can reach the NTFF profiler."""
    import types

    if "antenv.axon_hooks" in sys.modules:
        return
    import antenv

    mod = types.ModuleType("antenv.axon_hooks")
    state = {}
    mod.set_axon_ntff_profile_hook = lambda h: state.__setitem__("h", h)
    mod.get_axon_ntff_profile_hook = lambda: state.get("h")
    sys.modules["antenv.axon_hooks"] = mod
    antenv.axon_hooks = mod
    try:
        if "/root/.axon_site" not in sys.path:
            sys.path.insert(0, "/root/.axon_site")
        from trn_agent_boot.trn_boot import _ntff_profile_via_ctypes

        hook = _ntff_profile_via_ctypes("/opt/axon/libaxon_pjrt.so")
        if hook is not None:
            mod.set_axon_ntff_profile_hook(hook)
    except Exception:
        pass


def _run_on_hw(x, a, beta, trace=False):
    global LAST_RESULTS
    if trace:
        _ensure_ntff_hook()
    from concourse.bass_utils import run_bass_kernel_spmd

    # vertical banded matrix: V[i, j] = coeff of x-row i in t-row j
    V = np.zeros((H, H), dtype=np.float32)
    idx = np.arange(H)
    V[idx[:-1] + 1, idx[:-1]] += a[2]   # i = j+1
    V[idx, idx] += a[1]                 # i = j
    V[idx[1:] - 1, idx[1:]] += a[0]     # i = j-1

    key = (a.tobytes(), float(beta))
    if key not in _CACHE:
        _CACHE[key] = _build_bass(beta)
    nc = _CACHE[key]

    # host-side zero padding of one w column on each side (pads the slab so
    # the device needs no memsets)
    xp = np.zeros((N_CORES, ROWS, SLAB), dtype=np.float32)
    xp[:, :, PAD : PAD + FS] = x.reshape(N_CORES, ROWS, FS)
    in_maps = [{"x": xp[c], "vmat": V} for c in range(N_CORES)]
    res = run_bass_kernel_spmd(nc, in_maps, list(range(N_CORES)), trace=trace)
    LAST_RESULTS = res
    out = np.stack([res.results[c]["out"] for c in range(N_CORES)])
    return out.reshape(N, H, W, CH)


def kernel(x, kernel=None, _trace=False, **_unused):
    x = np.ascontiguousarray(np.asarray(x, dtype=np.float32))
    assert x.shape == (N, H, W, CH), f"unexpected x shape {x.shape}"
    if kernel is None:
        base = np.array(
            [[1.0, 0.0, -1.0], [0.0, 1.0, 0.0], [-1.0, 0.0, 1.0]], dtype=np.float32
        )
        kernel = np.tile(base[:, :, None, None], (1, 1, 1, CH))
    params = _stencil_params(kernel)
    if params is None:
        return _numpy_fallback(x, kernel)
    a, beta = params
    return _run_on_hw(x, a, beta, trace=_trace)


if __name__ == "__main__":
    xs = np.random.randn(N, H, W, CH).astype(np.float32)
    out = kernel(xs)
    print(out.shape, out.dtype)


# revision 10
# speedup vs baseline: 1.2996x; 1.2855x over previous
"""Trainium2 Bass kernel: depthwise 3x3 stencil conv (SAME, zero-pad) + residual.

Math (per image, per channel):
    out[h,w] = sum_{dh,dw} k[dh,dw] * x[h+dh-1, w+dw-1]  +  x[h,w]

The fixed stencil k = [[1,0,-1],[0,1,0],[-1,0,1]] is rank-2:
    k = outer((1,0,-1),(1,0,-1)) + center(1)
so with t[h,w] = x[h-1,w] - x[h+1,w] (vertical pass):
    out[h,w] = 2*x[h,w] + t[h,w-1] - t[h,w+1]

Mapping on one NeuronCore (batch is sharded 4 images/core across 8 cores):
  - layout: partitions = h (112 rows), free dim = (w,c) flattened (10752 f32)
    with 96-float zero pads on both ends (one w column, padded host-side)
  - vertical pass: banded 112x112 matmul on TensorE (PSUM, N=512 chunks)
  - PSUM -> SBUF t-slab copies on ScalarE
  - horizontal pass: two fused in-place DVE ops per piece:
        v   = 2*x + t@(w-1)      (scalar_tensor_tensor)
        out = v - t@(w+1)        (tensor_tensor)
  - straight contiguous HBM DMAs in/out (HWDGE)

fp32 self-loading matmuls can carry only ~1 semaphore wait (single EVENTS
slot in the LDWEIGHTS ISA struct), so DMA-completion waits are absorbed by
tiny dummy matmuls that read one column of the freshly loaded tile.
"""

import sys
import numpy as np

for _p in ("/opt/trn_rl_repo",):
    if _p not in sys.path:
        sys.path.insert(0, _p)

# ---------------- problem constants (hardcoded per contract) ----------------
N_CORES = 8
N, H, W, CH = 32, 112, 112, 96
IMGS_PER_CORE = N // N_CORES          # 4
ROWS = IMGS_PER_CORE * H              # 448 rows per core shard
FS = W * CH                           # 10752 floats per row
PAD = CH                              # one w column of zero padding
SLAB = FS + 2 * PAD                   # 10944
MM_N = 512                            # one PSUM bank of fp32
N_PIECES = 3                          # DVE piece split of the interior
PIECE = FS // N_PIECES                # 3584

_CACHE = {}
LAST_RESULTS = None  # BassKernelResults of the most recent run (for test.py)


def _build_bass(beta):
    """Raw-bass program with a hand-rolled static schedule.

    The walrus codegen used on this toolchain supports at most ONE semaphore
    wait per instruction, which rules out Tile's auto-generated multi-wait
    instructions.  Raw bass emits each wait as its own standalone wait_ge
    instruction on the consuming engine, which is always legal.

    Work is split into 8 units (4 images x 2 w-halves) with 4-deep slab
    buffering so load / matmul / copy / vector / store stages of different
    units overlap.  Per unit u:
        SP :  D(u)  x rows, w-halo cols -> xs[u%4]   (HBM -> SBUF, 2.4 MB)
        PE :  mm(u,g) ps[bank] = V^T @ xs[:, g]      (vertical pass, 11 groups)
        ACT:  cp(u,g) ts[u%4][:, g] <- ps[bank]      (PSUM -> SBUF)
        DVE:  op1  xs[96:5472] = beta*xs + ts[0:5376]        (v = 2x + t@w-1)
              op2  ts[96:5472] = xs[96:5472] - ts[192:5568]  (out = v - t@w+1)
              drain -> inc dve sem
        SP :  O(u)  ts[96:5472] -> out rows/cols     (SBUF -> HBM)
    """
    from concourse import bass, mybir

    f32 = mybir.dt.float32
    nc = bass.Bass(debug=False)
    x_d = nc.declare_dram_parameter("x", [ROWS, SLAB], f32, isOutput=False)
    v_d = nc.declare_dram_parameter("vmat", [H, H], f32, isOutput=False)
    out_d = nc.declare_dram_parameter("out", [ROWS, FS], f32, isOutput=True)

    WHALF = W // 2            # 56 output columns per unit
    USLAB = (WHALF + 2) * CH  # 5568 slab floats (1 w-col halo each side)
    UINT = WHALF * CH         # 5376 interior floats
    NU = IMGS_PER_CORE * 2    # 8 units
    NS = 4                    # slab sets in flight

    groups = []
    off = 0
    while off < USLAB:
        n = min(MM_N, USLAB - off)
        groups.append((off, n))
        off += n
    n_g = len(groups)  # 11

    vt = nc.alloc_sbuf_tensor("vt", [H, H], f32)
    xs = [nc.alloc_sbuf_tensor(f"xs{k}", [H, USLAB], f32) for k in range(NS)]
    ts = [nc.alloc_sbuf_tensor(f"ts{k}", [H, USLAB], f32) for k in range(NS)]
    NB = 8
    ps = [nc.alloc_psum_tensor(f"ps{b}", [H, MM_N], f32) for b in range(NB)]

    def unit_rows(u):
        i = u // 2
        return i * H, (i + 1) * H

    def unit_slab_col(u):
        # start column of the unit's slab inside the padded x row [ROWS, SLAB]
        return (u % 2) * WHALF * CH  # 0 or 5376

    from contextlib import ExitStack

    with (
        nc.Block(no_gpsimd_drain=True) as block,
        nc.semaphore("s_vt") as s_vt,
        nc.semaphore("s_pe") as s_pe,
        nc.semaphore("s_act") as s_act,
        nc.semaphore("s_dve") as s_dve,
        ExitStack() as _sems,
    ):
        # Per-slab-set DMA completion semaphores.  A single cumulative DMA
        # semaphore would race: concurrent DMAs can complete out of issue
        # order, so "sem >= 16*(u+1)" could be satisfied by a LATER unit's
        # transfer while unit u's data is still in flight.  Per-set sems are
        # safe because successive users of one set never overlap in flight.
        s_din = [_sems.enter_context(nc.semaphore(f"s_din{k}")) for k in range(NS)]
        s_dout = [_sems.enter_context(nc.semaphore(f"s_dout{k}")) for k in range(NS)]

        @block.sync
        def _(sp: bass.BassEngine):
            sp.dma_start(out=vt[:, :], in_=v_d[:, :]).then_inc(s_vt, 16)

            def load(u):
                r0, r1 = unit_rows(u)
                c0 = unit_slab_col(u)
                sp.dma_start(
                    out=xs[u % NS][:, :], in_=x_d[r0:r1, c0 : c0 + USLAB]
                ).then_inc(s_din[u % NS], 16)

            for u in range(min(NS, NU)):
                load(u)
            for u in range(NU):
                r0, r1 = unit_rows(u)
                oc0 = (u % 2) * UINT
                # store unit u once its DVE drain fired
                sp.wait_ge(s_dve, u + 1)
                sp.dma_start(
                    out=out_d[r0:r1, oc0 : oc0 + UINT],
                    in_=ts[u % NS][:, PAD : PAD + UINT],
                ).then_inc(s_dout[u % NS], 16)
                nxt = u + NS
                if nxt < NU:
                    # reload xs[u%NS]: PE reads of unit u must be done (DVE
                    # covered by the store wait above)
                    sp.wait_ge(s_pe, n_g * (u + 1))
                    load(nxt)
            for k in range(NS):
                sp.wait_ge(s_dout[k], 16 * (NU // NS))

        @block.tensor
        def _(pe: bass.BassEngine):
            pe.wait_ge(s_vt, 16)
            for u in range(NU):
                pe.wait_ge(s_din[u % NS], 16 * (u // NS + 1))
                for g, (goff, gn) in enumerate(groups):
                    idx = u * n_g + g
                    if idx >= NB:
                        # psum bank reuse: the copy that read it must be done
                        pe.wait_ge(s_act, idx - NB + 1)
                    pe.matmul(
                        out=ps[idx % NB][0:H, 0:gn],
                        lhsT=vt[:, :],
                        rhs=xs[u % NS][:, goff : goff + gn],
                        start=True,
                        stop=True,
                    ).then_inc(s_pe, 1)

        @block.scalar
        def _(act: bass.BassEngine):
            for u in range(NU):
                if u >= NS:
                    # ts slab reuse: unit u-NS's DVE write and store DMA done
                    act.wait_ge(s_dve, u - NS + 1)
                    act.wait_ge(s_dout[u % NS], 16 * (u // NS))
                for g, (goff, gn) in enumerate(groups):
                    idx = u * n_g + g
                    act.wait_ge(s_pe, idx + 1)
                    act.copy(
                        out=ts[u % NS][:, goff : goff + gn],
                        in_=ps[idx % NB][0:H, 0:gn],
                    ).then_inc(s_act, 1)

        @block.vector
        def _(dve: bass.BassEngine):
            for u in range(NU):
                # all matmul groups of unit u must have read xs before op1
                # overwrites it, and all copies must have produced ts
                dve.wait_ge(s_pe, n_g * (u + 1))
                dve.wait_ge(s_act, n_g * (u + 1))
                dve.scalar_tensor_tensor(
                    out=xs[u % NS][:, PAD : PAD + UINT],
                    in0=xs[u % NS][:, PAD : PAD + UINT],
                    scalar=float(beta),
                    in1=ts[u % NS][:, 0:UINT],
                    op0=mybir.AluOpType.mult,
                    op1=mybir.AluOpType.add,
                )
                dve.tensor_tensor(
                    out=ts[u % NS][:, PAD : PAD + UINT],
                    in0=xs[u % NS][:, PAD : PAD + UINT],
                    in1=ts[u % NS][:, 2 * PAD : 2 * PAD + UINT],
                    op=mybir.AluOpType.subtract,
                )
                dve.drain().then_inc(s_dve, 1)

    return nc


def _stencil_params(kern):
    """Validate the depthwise kernel and extract (vertical profile a, beta).

    Requires: channels identical, k[:,2] == -k[:,0], k[0,1] == k[2,1] == 0.
    Returns (a, beta) with a = k[:,0] (vertical mixing profile) and
    beta = k[1,1] + 1 (center coefficient incl. the residual).
    """
    k = np.asarray(kern, dtype=np.float32)
    if k.ndim != 4 or k.shape != (3, 3, 1, CH):
        return None
    if not np.all(k == k[:, :, :, :1]):
        return None
    k2 = k[:, :, 0, 0]
    if not (np.all(k2[:, 2] == -k2[:, 0]) and k2[0, 1] == 0 and k2[2, 1] == 0):
        return None
    return k2[:, 0].copy(), float(k2[1, 1]) + 1.0


def _numpy_fallback(x, kern):
    """Straightforward shifted-add implementation (safety net only)."""
    k = np.asarray(kern, dtype=np.float32)[:, :, 0, :]  # (3,3,CH)
    xp = np.pad(x, ((0, 0), (1, 1), (1, 1), (0, 0)))
    out = x.astype(np.float32).copy()
    for dh in range(3):
        for dw in range(3):
            out += k[dh, dw] * xp[:, dh : dh + H, dw : dw + W, :]
    return out


def _ensure_ntff_hook():
    """The agent image's antenv lacks axon_hooks; synthesize it so
    run_bass_kernel_spmd(trace=True) can reach the NTFF profiler."""
    import types

    if "antenv.axon_hooks" in sys.modules:
        return
    import antenv

    mod = types.ModuleType("antenv.axon_hooks")
    state = {}
    mod.set_axon_ntff_profile_hook = lambda h: state.__setitem__("h", h)
    mod.get_axon_ntff_profile_hook = lambda: state.get("h")
    sys.modules["antenv.axon_hooks"] = mod
    antenv.axon_hooks = mod
    try:
        if "/root/.axon_site" not in sys.path:
            sys.path.insert(0, "/root/.axon_site")
        from trn_agent_boot.trn_boot import _ntff_profile_via_ctypes

        hook = _ntff_profile_via_ctypes("/opt/axon/libaxon_pjrt.so")
        if hook is not None:
            mod.set_axon_ntff_profile_hook(hook)
    except Exception:
        pass


def _run_on_hw(x, a, beta, trace=False):
    global LAST_RESULTS
    if trace:
        _ensure_ntff_hook()
    from concourse.bass_utils import run_bass_kernel_spmd

    # vertical banded matrix: V[i, j] = coeff of x-row i in t-row j
    V = np.zeros((H, H), dtype=np.float32)
    idx = np.arange(H)
    V[idx[:-1] + 1, idx[:-1]] += a[2]   # i = j+1
    V[idx, idx] += a[1]                 # i = j
    V[idx[1:] - 1, idx[1:]] += a[0]     # i = j-1

    key = (a.tobytes(), float(beta))
    if key not in _CACHE:
        _CACHE[key] = _build_bass(beta)
    nc = _CACHE[key]

    # host-side zero padding of one w column on each side (pads the slab so
    # the device needs no memsets)
    xp = np.zeros((N_CORES, ROWS, SLAB), dtype=np.float32)
    xp[:, :, PAD : PAD + FS] = x.reshape(N_CORES, ROWS, FS)
    in_maps = [{"x": xp[c], "vmat": V} for c in range(N_CORES)]
    res = run_bass_kernel_spmd(nc, in_maps, list(range(N_CORES)), trace=trace)
    LAST_RESULTS = res
    out = np.stack([res.results[c]["out"] for c in range(N_CORES)])
    return out.reshape(N, H, W, CH)


def kernel(x, kernel=None, _trace=False, **_unused):
    x = np.ascontiguousarray(np.asarray(x, dtype=np.float32))
    assert x.shape == (N, H, W, CH), f"unexpected x shape {x.shape}"
    if kernel is None:
        base = np.array(
            [[1.0, 0.0, -1.0], [0.0, 1.0, 0.0], [-1.0, 0.0, 1.0]], dtype=np.float32
        )
        kernel = np.tile(base[:, :, None, None], (1, 1, 1, CH))
    params = _stencil_params(kernel)
    if params is None:
        return _numpy_fallback(x, kernel)
    a, beta = params
    return _run_on_hw(x, a, beta, trace=_trace)


if __name__ == "__main__":
    xs = np.random.randn(N, H, W, CH).astype(np.float32)
    out = kernel(xs)
    print(out.shape, out.dtype)


# revision 17
# speedup vs baseline: 1.3493x; 1.0382x over previous
"""Trainium2 Bass kernel: depthwise 3x3 stencil conv (SAME, zero-pad) + residual.

Math (per image, per channel):
    out[h,w] = sum_{dh,dw} k[dh,dw] * x[h+dh-1, w+dw-1]  +  x[h,w]

The fixed stencil k = [[1,0,-1],[0,1,0],[-1,0,1]] is rank-2:
    k = outer((1,0,-1),(1,0,-1)) + center(1)
so with t[h,w] = x[h-1,w] - x[h+1,w] (vertical pass):
    out[h,w] = 2*x[h,w] + t[h,w-1] - t[h,w+1]

Mapping on one NeuronCore (batch is sharded 4 images/core across 8 cores):
  - layout: partitions = h (112 rows), free dim = (w,c) flattened (10752 f32)
    with 96-float zero pads on both ends (one w column, padded host-side)
  - vertical pass: banded 112x112 matmul on TensorE (PSUM, N=512 chunks)
  - PSUM -> SBUF t-slab copies on ScalarE
  - horizontal pass: two fused in-place DVE ops per piece:
        v   = 2*x + t@(w-1)      (scalar_tensor_tensor)
        out = v - t@(w+1)        (tensor_tensor)
  - straight contiguous HBM DMAs in/out (HWDGE)

fp32 self-loading matmuls can carry only ~1 semaphore wait (single EVENTS
slot in the LDWEIGHTS ISA struct), so DMA-completion waits are absorbed by
tiny dummy matmuls that read one column of the freshly loaded tile.
"""

import sys
import numpy as np

for _p in ("/opt/trn_rl_repo",):
    if _p not in sys.path:
        sys.path.insert(0, _p)

# ---------------- problem constants (hardcoded per contract) ----------------
N_CORES = 8
N, H, W, CH = 32, 112, 112, 96
IMGS_PER_CORE = N // N_CORES          # 4
ROWS = IMGS_PER_CORE * H              # 448 rows per core shard
FS = W * CH                           # 10752 floats per row
PAD = CH                              # one w column of zero padding
SLAB = FS + 2 * PAD                   # 10944
MM_N = 512                            # one PSUM bank of fp32
N_PIECES = 3                          # DVE piece split of the interior
PIECE = FS // N_PIECES                # 3584

_CACHE = {}
LAST_RESULTS = None  # BassKernelResults of the most recent run (for test.py)


def _build_bass(beta):
    """Raw-bass program with a hand-rolled static schedule.

    The walrus codegen used on this toolchain supports at most ONE semaphore
    wait per instruction, which rules out Tile's auto-generated multi-wait
    instructions.  Raw bass emits each wait as its own standalone wait_ge
    instruction on the consuming engine, which is always legal.

    Work is split into 8 units (4 images x 2 w-halves) with 4-deep slab
    buffering so load / matmul / copy / vector / store stages of different
    units overlap.  Per unit u:
        SP :  D(u)  x rows, w-halo cols -> xs[u%4]   (HBM -> SBUF, 2.4 MB)
        PE :  per group g, two accumulating float32r matmuls produce the
              whole shifted-stencil term in PSUM:
                ps[bank] = V^T @ xs[:, g+0]  +  (-V)^T @ xs[:, g+192]
                         = t@(w-1) - t@(w+1)
        ACT:  cp(u,g) ts[u%4][:, g] <- ps[bank]      (PSUM -> SBUF)
        DVE:  op   ts[0:5376] = beta * xs[96:5472] + ts[0:5376]   (exact fp32)
              drain -> inc dve sem
        SP :  O(u)  ts[0:5376] -> out rows/cols      (SBUF -> HBM)

    The matmuls run in float32r (single-pass fp32, ~1e-4 relative error on
    the stencil term, 4x faster than strict fp32 on the PE); the dominant
    residual/center term beta*x stays exact fp32 on the DVE.
    """
    from concourse import bass, mybir

    f32 = mybir.dt.float32
    f32r = mybir.dt.float32r
    nc = bass.Bass(debug=False)
    x_d = nc.declare_dram_parameter("x", [ROWS, SLAB], f32r, isOutput=False)
    v_d = nc.declare_dram_parameter("vmat", [H, H], f32r, isOutput=False)
    vn_d = nc.declare_dram_parameter("vmatn", [H, H], f32r, isOutput=False)
    out_d = nc.declare_dram_parameter("out", [ROWS, FS], f32, isOutput=True)

    WHALF = W // 2            # 56 output columns per unit
    USLAB = (WHALF + 2) * CH  # 5568 slab floats (1 w-col halo each side)
    UINT = WHALF * CH         # 5376 interior floats
    NU = IMGS_PER_CORE * 2    # 8 units
    NS = 4                    # slab sets in flight

    groups = []
    off = 0
    while off < UINT:
        n = min(MM_N, UINT - off)
        groups.append((off, n))
        off += n
    n_g = len(groups)  # 11 (10x512 + 256)

    vt = nc.alloc_sbuf_tensor("vt", [H, H], f32r)
    vtn = nc.alloc_sbuf_tensor("vtn", [H, H], f32r)
    xs = [nc.alloc_sbuf_tensor(f"xs{k}", [H, USLAB], f32r) for k in range(NS)]
    ts = [nc.alloc_sbuf_tensor(f"ts{k}", [H, USLAB], f32) for k in range(NS)]
    NB = 8
    ps = [nc.alloc_psum_tensor(f"ps{b}", [H, MM_N], f32) for b in range(NB)]

    def unit_rows(u):
        i = u // 2
        return i * H, (i + 1) * H

    def unit_slab_col(u):
        # start column of the unit's slab inside the padded x row [ROWS, SLAB]
        return (u % 2) * WHALF * CH  # 0 or 5376

    from contextlib import ExitStack

    with (
        nc.Block(no_gpsimd_drain=True) as block,
        nc.semaphore("s_vt") as s_vt,
        nc.semaphore("s_pe") as s_pe,
        nc.semaphore("s_act") as s_act,
        nc.semaphore("s_dve") as s_dve,
        ExitStack() as _sems,
    ):
        # Per-slab-set DMA completion semaphores.  A single cumulative DMA
        # semaphore would race: concurrent DMAs can complete out of issue
        # order, so "sem >= 16*(u+1)" could be satisfied by a LATER unit's
        # transfer while unit u's data is still in flight.  Per-set sems are
        # safe because successive users of one set never overlap in flight.
        s_din = [_sems.enter_context(nc.semaphore(f"s_din{k}")) for k in range(NS)]
        s_dout = [_sems.enter_context(nc.semaphore(f"s_dout{k}")) for k in range(NS)]

        @block.sync
        def _(sp: bass.BassEngine):
            sp.dma_start(out=vt[:, :], in_=v_d[:, :]).then_inc(s_vt, 16)
            sp.dma_start(out=vtn[:, :], in_=vn_d[:, :]).then_inc(s_vt, 16)

            def load(u):
                r0, r1 = unit_rows(u)
                c0 = unit_slab_col(u)
                sp.dma_start(
                    out=xs[u % NS][:, :], in_=x_d[r0:r1, c0 : c0 + USLAB]
                ).then_inc(s_din[u % NS], 16)

            for u in range(min(NS, NU)):
                load(u)
            for u in range(NU):
                r0, r1 = unit_rows(u)
                oc0 = (u % 2) * UINT
                # store unit u once its DVE drain fired
                sp.wait_ge(s_dve, u + 1)
                sp.dma_start(
                    out=out_d[r0:r1, oc0 : oc0 + UINT],
                    in_=ts[u % NS][:, 0:UINT],
                ).then_inc(s_dout[u % NS], 16)
                nxt = u + NS
                if nxt < NU:
                    # reload xs[u%NS]: PE reads of unit u must be done (DVE
                    # covered by the store wait above)
                    sp.wait_ge(s_pe, n_g * (u + 1))
                    load(nxt)
            for k in range(NS):
                sp.wait_ge(s_dout[k], 16 * (NU // NS))

        @block.tensor
        def _(pe: bass.BassEngine):
            pe.wait_ge(s_vt, 32)
            for u in range(NU):
                pe.wait_ge(s_din[u % NS], 16 * (u // NS + 1))
                for g, (goff, gn) in enumerate(groups):
                    idx = u * n_g + g
                    if idx >= NB:
                        # psum bank reuse: the copy that read it must be done
                        pe.wait_ge(s_act, idx - NB + 1)
                    # ps = V^T @ x(w-1)  -  V^T @ x(w+1)  =  t@(w-1) - t@(w+1)
                    pe.matmul(
                        out=ps[idx % NB][0:H, 0:gn],
                        lhsT=vt[:, :],
                        rhs=xs[u % NS][:, goff : goff + gn],
                        start=True,
                        stop=False,
                    )
                    pe.matmul(
                        out=ps[idx % NB][0:H, 0:gn],
                        lhsT=vtn[:, :],
                        rhs=xs[u % NS][:, goff + 2 * PAD : goff + 2 * PAD + gn],
                        start=False,
                        stop=True,
                    ).then_inc(s_pe, 1)

        @block.scalar
        def _(act: bass.BassEngine):
            for u in range(NU):
                if u >= NS:
                    # ts slab reuse: unit u-NS's DVE write and store DMA done
                    act.wait_ge(s_dve, u - NS + 1)
                    act.wait_ge(s_dout[u % NS], 16 * (u // NS))
                for g, (goff, gn) in enumerate(groups):
                    idx = u * n_g + g
                    act.wait_ge(s_pe, idx + 1)
                    act.copy(
                        out=ts[u % NS][:, goff : goff + gn],
                        in_=ps[idx % NB][0:H, 0:gn],
                    ).then_inc(s_act, 1)

        @block.vector
        def _(dve: bass.BassEngine):
            for u in range(NU):
                # all copies must have produced ts (transitively: matmuls and
                # the load are done too; the op only READS xs)
                dve.wait_ge(s_act, n_g * (u + 1))
                dve.scalar_tensor_tensor(
                    out=ts[u % NS][:, 0:UINT],
                    in0=xs[u % NS][:, PAD : PAD + UINT].bitcast(f32),
                    scalar=float(beta),
                    in1=ts[u % NS][:, 0:UINT],
                    op0=mybir.AluOpType.mult,
                    op1=mybir.AluOpType.add,
                )
                dve.drain().then_inc(s_dve, 1)

    return nc


def _stencil_params(kern):
    """Validate the depthwise kernel and extract (vertical profile a, beta).

    Requires: channels identical, k[:,2] == -k[:,0], k[0,1] == k[2,1] == 0.
    Returns (a, beta) with a = k[:,0] (vertical mixing profile) and
    beta = k[1,1] + 1 (center coefficient incl. the residual).
    """
    k = np.asarray(kern, dtype=np.float32)
    if k.ndim != 4 or k.shape != (3, 3, 1, CH):
        return None
    if not np.all(k == k[:, :, :, :1]):
        return None
    k2 = k[:, :, 0, 0]
    if not (np.all(k2[:, 2] == -k2[:, 0]) and k2[0, 1] == 0 and k2[2, 1] == 0):
        return None
    return k2[:, 0].copy(), float(k2[1, 1]) + 1.0


def _numpy_fallback(x, kern):
    """Straightforward shifted-add implementation (safety net only)."""
    k = np.asarray(kern, dtype=np.float32)[:, :, 0, :]  # (3,3,CH)
    xp = np.pad(x, ((0, 0), (1, 1), (1, 1), (0, 0)))
    out = x.astype(np.float32).copy()
    for dh in range(3):
        for dw in range(3):
            out += k[dh, dw] * xp[:, dh : dh + H, dw : dw + W, :]
    return out


def _ensure_ntff_hook():
    """The agent image's antenv lacks axon_hooks; synthesize it so
    run_bass_kernel_spmd(trace=True) can reach the NTFF profiler."""
    import types

    if "antenv.axon_hooks" in sys.modules:
        return
    import antenv

    mod = types.ModuleType("antenv.axon_hooks")
    state = {}
    mod.set_axon_ntff_profile_hook = lambda h: state.__setitem__("h", h)
    mod.get_axon_ntff_profile_hook = lambda: state.get("h")
    sys.modules["antenv.axon_hooks"] = mod
    antenv.axon_hooks = mod
    try:
        if "/root/.axon_site" not in sys.path:
            sys.path.insert(0, "/root/.axon_site")
        from trn_agent_boot.trn_boot import _ntff_profile_via_ctypes

        hook = _ntff_profile_via_ctypes("/opt/axon/libaxon_pjrt.so")
        if hook is not None:
            mod.set_axon_ntff_profile_hook(hook)
    except Exception:
        pass


def _run_on_hw(x, a, beta, trace=False):
    global LAST_RESULTS
    if trace:
        _ensure_ntff_hook()
    from concourse.bass_utils import run_bass_kernel_spmd

    # vertical banded matrix: V[i, j] = coeff of x-row i in t-row j
    V = np.zeros((H, H), dtype=np.float32)
    idx = np.arange(H)
    V[idx[:-1] + 1, idx[:-1]] += a[2]   # i = j+1
    V[idx, idx] += a[1]                 # i = j
    V[idx[1:] - 1, idx[1:]] += a[0]     # i = j-1

    key = (a.tobytes(), float(beta))
    if key not in _CACHE:
        _CACHE[key] = _build_bass(beta)
    nc = _CACHE[key]

    # host-side zero padding of one w column on each side (pads the slab so
    # the device needs no memsets)
    xp = np.zeros((N_CORES, ROWS, SLAB), dtype=np.float32)
    xp[:, :, PAD : PAD + FS] = x.reshape(N_CORES, ROWS, FS)
    Vn = np.ascontiguousarray(-V)
    in_maps = [{"x": xp[c], "vmat": V, "vmatn": Vn} for c in range(N_CORES)]
    res = run_bass_kernel_spmd(nc, in_maps, list(range(N_CORES)), trace=trace)
    LAST_RESULTS = res
    out = np.stack([res.results[c]["out"] for c in range(N_CORES)])
    return out.reshape(N, H, W, CH)


def kernel(x, kernel=None, _trace=False, **_unused):
    x = np.ascontiguousarray(np.asarray(x, dtype=np.float32))
    assert x.shape == (N, H, W, CH), f"unexpected x shape {x.shape}"
    if kernel is None:
        base = np.array(
            [[1.0, 0.0, -1.0], [0.0, 1.0, 0.0], [-1.0, 0.0, 1.0]], dtype=np.float32
        )
        kernel = np.tile(base[:, :, None, None], (1, 1, 1, CH))
    params = _stencil_params(kernel)
    if params is None:
        return _numpy_fallback(x, kernel)
    a, beta = params
    return _run_on_hw(x, a, beta, trace=_trace)


if __name__ == "__main__":
    xs = np.random.randn(N, H, W, CH).astype(np.float32)
    out = kernel(xs)
    print(out.shape, out.dtype)
